# revision 5
# baseline (speedup 1.0000x reference)
"""Chf (characteristic-function) loss kernel for Trainium2, SPMD over 8 cores.

Math: the reference builds cos/sin templates over a (u,v) frequency grid and
an N = W*H pixel grid with angle[u,v,(w,h)] = freq[v]*x[w] + freq[u]*y[h],
then contracts against the flattened image. Because the angle is separable,
cos/sin addition formulas factor the contraction into per-axis pieces:

  chf_real[b,u,v] = sum_{h,w} (Cx[v,w]*Cy[u,h] - Sx[v,w]*Sy[u,h]) * D[b,h,w]
  chf_img [b,u,v] = sum_{h,w} (Sx[v,w]*Cy[u,h] + Cx[v,w]*Sy[u,h]) * D[b,h,w]

with Cx[v,w] = cos(freq[v]*x[w]) etc. So per batch it is two 128x128x128
GEMM stages instead of a (4096 x 16384) template GEMM plus 134M cos/sin
evaluations. Sharding: data-parallel, 2 batches per core; each core emits
per-batch sum-of-squares of (derived - chf); host does sqrt/scale/mean.
"""

import os
import sys

import numpy as np

for _p in ("/opt/trn_rl_repo", "/root/.axon_site/_ro/trn_rl_repo"):
    if os.path.isdir(_p) and _p not in sys.path:
        sys.path.insert(0, _p)

from concourse import bacc, bass, mybir, tile  # noqa: E402
from concourse.bass_utils import run_bass_kernel_spmd  # noqa: E402

CHF_STEP = 32
CHF_TIK = 0.05
SAMPLE_STEP = 1.0
B, H, W = 16, 128, 128
S2 = 2 * CHF_STEP  # 64
N_CORES = 8
BPC = B // N_CORES  # batches per core
F32 = mybir.dt.float32


def _trig_constants():
    x = SAMPLE_STEP / 2 + SAMPLE_STEP * np.arange(W, dtype=np.float64)
    y = SAMPLE_STEP / 2 + SAMPLE_STEP * np.arange(H, dtype=np.float64)
    freq = np.arange(-CHF_STEP, CHF_STEP, dtype=np.float64) * CHF_TIK
    angx = x[:, None] * freq[None, :]  # (W, S2)
    angy = y[:, None] * freq[None, :]  # (H, S2)
    cxt, sxt = np.cos(angx), np.sin(angx)  # CxT[w,v], SxT[w,v]
    cyt, syt = np.cos(angy), np.sin(angy)  # CyT[h,u], SyT[h,u]
    trig_y = np.ascontiguousarray(
        np.concatenate([cyt, syt], axis=1).astype(np.float32)
    )  # (H, 2*S2) = [CyT | SyT]
    ones = np.ones((W, 1), dtype=np.float64)
    trig_x = np.ascontiguousarray(
        np.concatenate([cxt, sxt, -sxt, cxt, ones], axis=1).astype(np.float32)
    )  # (W, 4*S2+1) = [CxT | SxT | -SxT | CxT | 1]
    return trig_y, trig_x


def _build_nc():
    nc = bacc.Bacc("TRN2", target_bir_lowering=False, debug=False)
    dnn = nc.dram_tensor("dnn", [BPC, H, W], F32, kind="ExternalInput")
    chf = nc.dram_tensor("chf", [BPC, S2, S2, 2], F32, kind="ExternalInput")
    trig_y = nc.dram_tensor("trig_y", [H, 2 * S2], F32, kind="ExternalInput")
    trig_x = nc.dram_tensor("trig_x", [W, 4 * S2 + 1], F32, kind="ExternalInput")
    ssq = nc.dram_tensor("ssq", [1, BPC], F32, kind="ExternalOutput")

    with tile.TileContext(nc) as tc:
        with (
            tc.tile_pool(name="const", bufs=1) as cpool,
            tc.tile_pool(name="work", bufs=2) as wpool,
            tc.tile_pool(name="psum", bufs=2, space="PSUM") as ppool,
        ):
            ty = cpool.tile([H, 2 * S2], F32)
            nc.sync.dma_start(ty[:], trig_y[:])
            tx = cpool.tile([W, 4 * S2 + 1], F32)
            nc.sync.dma_start(tx[:], trig_x[:])
            cols = cpool.tile([S2, BPC], F32)

            for b in range(BPC):
                d = wpool.tile([H, W], F32, tag="d")
                nc.sync.dma_start(d[:], dnn[b])
                # stage 1: p1[w, :] = [sum_h D[h,w]*CyT[h,u] | sum_h D[h,w]*SyT[h,u]]
                p1 = ppool.tile([W, 2 * S2], F32, tag="p1")
                nc.tensor.matmul(p1[:], d[:], ty[:], start=True, stop=True)
                p1s = wpool.tile([W, 2 * S2], F32, tag="p1s")
                nc.vector.tensor_copy(p1s[:], p1[:])
                # stage 2: p2[u, :] = [real[u,v] | img[u,v]]
                #   += P1_c.T @ [CxT | SxT]   (start)
                #   += P1_s.T @ [-SxT | CxT]  (accumulate)
                p2 = ppool.tile([S2, 2 * S2], F32, tag="p2")
                nc.tensor.matmul(
                    p2[:], p1s[:, 0:S2], tx[:, 0 : 2 * S2], start=True, stop=False
                )
                nc.tensor.matmul(
                    p2[:], p1s[:, S2 : 2 * S2], tx[:, 2 * S2 : 4 * S2],
                    start=False, stop=True,
                )
                # matching chf slab: cht[u, 0:S2] = chf[b,u,v,0], cht[u, S2:] = chf[b,u,v,1]
                cht = wpool.tile([S2, 2 * S2], F32, tag="cht")
                nc.sync.dma_start(cht[:, 0:S2], chf[b, :, :, 0])
                nc.sync.dma_start(cht[:, S2 : 2 * S2], chf[b, :, :, 1])
                diff = wpool.tile([S2, 2 * S2], F32, tag="diff")
                nc.vector.tensor_sub(diff[:], p2[:], cht[:])
                sq = wpool.tile([S2, 2 * S2], F32, tag="sq")
                nc.scalar.activation(
                    sq[:],
                    diff[:],
                    mybir.ActivationFunctionType.Square,
                    accum_out=cols[:, b : b + 1],
                )

            # cross-partition reduce of per-batch partials: ssq[0,b] = sum_u cols[u,b]
            pss = ppool.tile([1, BPC], F32, tag="pss")
            nc.tensor.matmul(
                pss[:], tx[0:S2, 4 * S2 : 4 * S2 + 1], cols[:], start=True, stop=True
            )
            outt = cpool.tile([1, BPC], F32)
            nc.vector.tensor_copy(outt[:], pss[:])
            nc.sync.dma_start(ssq[:], outt[:])

    nc.compile()
    return nc


_NC_CACHE = None


def _get_nc():
    global _NC_CACHE
    if _NC_CACHE is None:
        _NC_CACHE = _build_nc()
    return _NC_CACHE


def kernel(dnn_output: np.ndarray, chf: np.ndarray) -> np.ndarray:
    dnn_output = np.ascontiguousarray(dnn_output, dtype=np.float32)
    chf = np.ascontiguousarray(chf, dtype=np.float32)
    trig_y, trig_x = _trig_constants()
    in_maps = [
        {
            "dnn": dnn_output[c * BPC : (c + 1) * BPC],
            "chf": chf[c * BPC : (c + 1) * BPC],
            "trig_y": trig_y,
            "trig_x": trig_x,
        }
        for c in range(N_CORES)
    ]
    nc = _get_nc()
    results = run_bass_kernel_spmd(nc, in_maps, list(range(N_CORES))).results
    ssq = np.concatenate([np.asarray(r["ssq"]).reshape(-1) for r in results])
    loss = np.sqrt(ssq.astype(np.float64)).sum() * CHF_TIK / B
    return np.float32(loss)


# revision 8
# speedup vs baseline: 1.4396x; 1.4396x over previous
"""Chf (characteristic-function) loss kernel for Trainium2, SPMD over 8 cores.

Math: the reference builds cos/sin templates over a (u,v) frequency grid and
an N = W*H pixel grid with angle[u,v,(w,h)] = freq[v]*x[w] + freq[u]*y[h],
then contracts against the flattened image. Because the angle is separable,
cos/sin addition formulas factor the contraction into per-axis pieces:

  chf_real[b,u,v] = sum_{h,w} (Cx[v,w]*Cy[u,h] - Sx[v,w]*Sy[u,h]) * D[b,h,w]
  chf_img [b,u,v] = sum_{h,w} (Sx[v,w]*Cy[u,h] + Cx[v,w]*Sy[u,h]) * D[b,h,w]

with Cx[v,w] = cos(freq[v]*x[w]) etc. So per batch it is two 128x128x128
GEMM stages instead of a (4096 x 16384) template GEMM plus 134M cos/sin
evaluations. Sharding: data-parallel, 2 batches per core; each core emits
per-batch sum-of-squares of (derived - chf); host does sqrt/scale/mean.
"""

import os
import sys

import numpy as np

for _p in ("/opt/trn_rl_repo", "/root/.axon_site/_ro/trn_rl_repo"):
    if os.path.isdir(_p) and _p not in sys.path:
        sys.path.insert(0, _p)

from concourse import bacc, bass, mybir, tile  # noqa: E402
from concourse.bass_utils import run_bass_kernel_spmd  # noqa: E402

CHF_STEP = 32
CHF_TIK = 0.05
SAMPLE_STEP = 1.0
B, H, W = 16, 128, 128
S2 = 2 * CHF_STEP  # 64
N_CORES = 8
BPC = B // N_CORES  # batches per core
F32 = mybir.dt.float32


def _trig_constants():
    x = SAMPLE_STEP / 2 + SAMPLE_STEP * np.arange(W, dtype=np.float64)
    y = SAMPLE_STEP / 2 + SAMPLE_STEP * np.arange(H, dtype=np.float64)
    freq = np.arange(-CHF_STEP, CHF_STEP, dtype=np.float64) * CHF_TIK
    angx = x[:, None] * freq[None, :]  # (W, S2)
    angy = y[:, None] * freq[None, :]  # (H, S2)
    cxt, sxt = np.cos(angx), np.sin(angx)  # CxT[w,v], SxT[w,v]
    cyt, syt = np.cos(angy), np.sin(angy)  # CyT[h,u], SyT[h,u]
    trig_y = np.ascontiguousarray(
        np.concatenate([cyt, syt], axis=1).astype(np.float32)
    )  # (H, 2*S2) = [CyT | SyT]
    ones = np.ones((W, 1), dtype=np.float64)
    # single constant slab: [CyT | SyT | CxT | SxT | -SxT | CxT | 1]  (H == W here)
    trig = np.ascontiguousarray(
        np.concatenate([cyt, syt, cxt, sxt, -sxt, cxt, ones], axis=1).astype(
            np.float32
        )
    )  # (128, 6*S2+1 = 385)
    return trig


def _build_nc():
    nc = bacc.Bacc("TRN2", target_bir_lowering=False, debug=False)
    dnn = nc.dram_tensor("dnn", [BPC, H, W], F32, kind="ExternalInput")
    chf = nc.dram_tensor("chf", [BPC, S2, S2, 2], F32, kind="ExternalInput")
    trig = nc.dram_tensor("trig", [H, 6 * S2 + 1], F32, kind="ExternalInput")
    ssq = nc.dram_tensor("ssq", [1, BPC], F32, kind="ExternalOutput")

    with tile.TileContext(nc) as tc:
        with (
            tc.tile_pool(name="const", bufs=1) as cpool,
            tc.tile_pool(name="work", bufs=2) as wpool,
            tc.tile_pool(name="psum", bufs=2, space="PSUM") as ppool,
        ):
            # three contiguous input DMAs on three different issuing queues
            tg = cpool.tile([H, 6 * S2 + 1], F32)
            nc.sync.dma_start(tg[:], trig[:])
            d_all = cpool.tile([H, BPC, W], F32)
            nc.gpsimd.dma_start(d_all[:], dnn.rearrange("b h w -> h b w"))
            cht = cpool.tile([S2, BPC, S2, 2], F32)
            nc.scalar.dma_start(cht[:], chf.rearrange("b u v c -> u b v c"))
            cols = cpool.tile([S2, BPC], F32)

            for b in range(BPC):
                # stage 1: p1[w, :] = [sum_h D[h,w]*CyT[h,u] | sum_h D[h,w]*SyT[h,u]]
                p1 = ppool.tile([W, 2 * S2], F32, tag="p1")
                nc.tensor.matmul(
                    p1[:], d_all[:, b, :], tg[:, 0 : 2 * S2], start=True, stop=True
                )
                p1s = wpool.tile([W, 2 * S2], F32, tag="p1s")
                nc.vector.tensor_copy(p1s[:], p1[:])
                # stage 2: p2[u, :] = [real[u,v] | img[u,v]]
                #   += P1_c.T @ [CxT | SxT]   (start)
                #   += P1_s.T @ [-SxT | CxT]  (accumulate)
                p2 = ppool.tile([S2, 2 * S2], F32, tag="p2")
                nc.tensor.matmul(
                    p2[:], p1s[:, 0:S2], tg[:, 2 * S2 : 4 * S2], start=True, stop=False
                )
                nc.tensor.matmul(
                    p2[:], p1s[:, S2 : 2 * S2], tg[:, 4 * S2 : 6 * S2],
                    start=False, stop=True,
                )
                # diff[u, c, v] = p2[u, c*S2+v] - chf[b, u, v, c]
                diff = wpool.tile([S2, 2, S2], F32, tag="diff")
                nc.vector.tensor_sub(
                    diff[:],
                    p2[:].rearrange("u (c v) -> u c v", c=2),
                    cht[:, b, :, :].rearrange("u v c -> u c v"),
                )
                sq = wpool.tile([S2, 2, S2], F32, tag="sq")
                nc.scalar.activation(
                    sq[:],
                    diff[:],
                    mybir.ActivationFunctionType.Square,
                    accum_out=cols[:, b : b + 1],
                )

            # cross-partition reduce of per-batch partials: ssq[0,b] = sum_u cols[u,b]
            pss = ppool.tile([1, BPC], F32, tag="pss")
            nc.tensor.matmul(
                pss[:], tg[0:S2, 6 * S2 : 6 * S2 + 1], cols[:], start=True, stop=True
            )
            outt = cpool.tile([1, BPC], F32)
            nc.vector.tensor_copy(outt[:], pss[:])
            nc.sync.dma_start(ssq[:], outt[:])

    nc.compile()
    return nc


_NC_CACHE = None


def _get_nc():
    global _NC_CACHE
    if _NC_CACHE is None:
        _NC_CACHE = _build_nc()
    return _NC_CACHE


def kernel(dnn_output: np.ndarray, chf: np.ndarray) -> np.ndarray:
    dnn_output = np.ascontiguousarray(dnn_output, dtype=np.float32)
    chf = np.ascontiguousarray(chf, dtype=np.float32)
    trig = _trig_constants()
    in_maps = [
        {
            "dnn": dnn_output[c * BPC : (c + 1) * BPC],
            "chf": chf[c * BPC : (c + 1) * BPC],
            "trig": trig,
        }
        for c in range(N_CORES)
    ]
    nc = _get_nc()
    results = run_bass_kernel_spmd(nc, in_maps, list(range(N_CORES))).results
    ssq = np.concatenate([np.asarray(r["ssq"]).reshape(-1) for r in results])
    loss = np.sqrt(ssq.astype(np.float64)).sum() * CHF_TIK / B
    return np.float32(loss)


# revision 10
# speedup vs baseline: 1.4763x; 1.0255x over previous
"""Chf (characteristic-function) loss kernel for Trainium2, SPMD over 8 cores.

Math: the reference builds cos/sin templates over a (u,v) frequency grid and
an N = W*H pixel grid with angle[u,v,(w,h)] = freq[v]*x[w] + freq[u]*y[h],
then contracts against the flattened image. Because the angle is separable,
cos/sin addition formulas factor the contraction into per-axis pieces:

  chf_real[b,u,v] = sum_{h,w} (Cx[v,w]*Cy[u,h] - Sx[v,w]*Sy[u,h]) * D[b,h,w]
  chf_img [b,u,v] = sum_{h,w} (Sx[v,w]*Cy[u,h] + Cx[v,w]*Sy[u,h]) * D[b,h,w]

with Cx[v,w] = cos(freq[v]*x[w]) etc. So per batch it is two 128x128x128
GEMM stages instead of a (4096 x 16384) template GEMM plus 134M cos/sin
evaluations. Sharding: data-parallel, 2 batches per core; each core emits
per-batch sum-of-squares of (derived - chf); host does sqrt/scale/mean.
"""

import os
import sys

import numpy as np

for _p in ("/opt/trn_rl_repo", "/root/.axon_site/_ro/trn_rl_repo"):
    if os.path.isdir(_p) and _p not in sys.path:
        sys.path.insert(0, _p)

from concourse import bacc, bass, mybir, tile  # noqa: E402
from concourse.bass_utils import run_bass_kernel_spmd  # noqa: E402

CHF_STEP = 32
CHF_TIK = 0.05
SAMPLE_STEP = 1.0
B, H, W = 16, 128, 128
S2 = 2 * CHF_STEP  # 64
N_CORES = 8
BPC = B // N_CORES  # batches per core
F32 = mybir.dt.float32


def _trig_constants():
    x = SAMPLE_STEP / 2 + SAMPLE_STEP * np.arange(W, dtype=np.float64)
    y = SAMPLE_STEP / 2 + SAMPLE_STEP * np.arange(H, dtype=np.float64)
    freq = np.arange(-CHF_STEP, CHF_STEP, dtype=np.float64) * CHF_TIK
    angx = x[:, None] * freq[None, :]  # (W, S2)
    angy = y[:, None] * freq[None, :]  # (H, S2)
    cxt, sxt = np.cos(angx), np.sin(angx)  # CxT[w,v], SxT[w,v]
    cyt, syt = np.cos(angy), np.sin(angy)  # CyT[h,u], SyT[h,u]
    trig_y = np.ascontiguousarray(
        np.concatenate([cyt, syt], axis=1).astype(np.float32)
    )  # (H, 2*S2) = [CyT | SyT]
    ones = np.ones((W, 1), dtype=np.float64)
    # single constant slab: [CyT | SyT | CxT | SxT | -SxT | CxT | 1]  (H == W here)
    trig = np.ascontiguousarray(
        np.concatenate([cyt, syt, cxt, sxt, -sxt, cxt, ones], axis=1).astype(
            np.float32
        )
    )  # (128, 6*S2+1 = 385)
    return trig


def _build_nc():
    nc = bacc.Bacc("TRN2", target_bir_lowering=False, debug=False)
    dnn = nc.dram_tensor("dnn", [BPC, H, W], F32, kind="ExternalInput")
    chf = nc.dram_tensor("chf", [BPC, S2, S2, 2], F32, kind="ExternalInput")
    trig = nc.dram_tensor("trig", [H, 6 * S2 + 1], F32, kind="ExternalInput")
    ssq = nc.dram_tensor("ssq", [1, BPC], F32, kind="ExternalOutput")

    with tile.TileContext(nc) as tc:
        with (
            tc.tile_pool(name="const", bufs=1) as cpool,
            tc.tile_pool(name="work", bufs=2) as wpool,
            tc.tile_pool(name="psum", bufs=2, space="PSUM") as ppool,
        ):
            # three contiguous input DMAs on three different issuing queues
            tg = cpool.tile([H, 6 * S2 + 1], F32)
            nc.sync.dma_start(tg[:], trig[:])
            d_all = cpool.tile([H, BPC, W], F32)
            nc.scalar.dma_start(d_all[:], dnn.rearrange("b h w -> h b w"))
            cht = cpool.tile([S2, BPC, S2, 2], F32)
            nc.scalar.dma_start(cht[:], chf.rearrange("b u v c -> u b v c"))
            cols = cpool.tile([S2, BPC], F32)

            for b in range(BPC):
                # stage 1: p1[w, :] = [sum_h D[h,w]*CyT[h,u] | sum_h D[h,w]*SyT[h,u]]
                p1 = ppool.tile([W, 2 * S2], F32, tag="p1")
                nc.tensor.matmul(
                    p1[:], d_all[:, b, :], tg[:, 0 : 2 * S2], start=True, stop=True
                )
                p1s = wpool.tile([W, 2 * S2], F32, tag="p1s")
                nc.vector.tensor_copy(p1s[:], p1[:])
                # stage 2: p2[u, :] = [real[u,v] | img[u,v]]
                #   += P1_c.T @ [CxT | SxT]   (start)
                #   += P1_s.T @ [-SxT | CxT]  (accumulate)
                p2 = ppool.tile([S2, 2 * S2], F32, tag="p2")
                nc.tensor.matmul(
                    p2[:], p1s[:, 0:S2], tg[:, 2 * S2 : 4 * S2], start=True, stop=False
                )
                nc.tensor.matmul(
                    p2[:], p1s[:, S2 : 2 * S2], tg[:, 4 * S2 : 6 * S2],
                    start=False, stop=True,
                )
                # diff[u, c, v] = p2[u, c*S2+v] - chf[b, u, v, c]
                diff = wpool.tile([S2, 2, S2], F32, tag="diff")
                nc.vector.tensor_sub(
                    diff[:],
                    p2[:].rearrange("u (c v) -> u c v", c=2),
                    cht[:, b, :, :].rearrange("u v c -> u c v"),
                )
                sq = wpool.tile([S2, 2, S2], F32, tag="sq")
                nc.scalar.activation(
                    sq[:],
                    diff[:],
                    mybir.ActivationFunctionType.Square,
                    accum_out=cols[:, b : b + 1],
                )

            # cross-partition reduce of per-batch partials: ssq[0,b] = sum_u cols[u,b]
            pss = ppool.tile([1, BPC], F32, tag="pss")
            nc.tensor.matmul(
                pss[:], tg[0:S2, 6 * S2 : 6 * S2 + 1], cols[:], start=True, stop=True
            )
            outt = cpool.tile([1, BPC], F32)
            nc.vector.tensor_copy(outt[:], pss[:])
            nc.sync.dma_start(ssq[:], outt[:])

    nc.compile()
    return nc


_NC_CACHE = None


def _get_nc():
    global _NC_CACHE
    if _NC_CACHE is None:
        _NC_CACHE = _build_nc()
    return _NC_CACHE


def kernel(dnn_output: np.ndarray, chf: np.ndarray) -> np.ndarray:
    dnn_output = np.ascontiguousarray(dnn_output, dtype=np.float32)
    chf = np.ascontiguousarray(chf, dtype=np.float32)
    trig = _trig_constants()
    in_maps = [
        {
            "dnn": dnn_output[c * BPC : (c + 1) * BPC],
            "chf": chf[c * BPC : (c + 1) * BPC],
            "trig": trig,
        }
        for c in range(N_CORES)
    ]
    nc = _get_nc()
    results = run_bass_kernel_spmd(nc, in_maps, list(range(N_CORES))).results
    ssq = np.concatenate([np.asarray(r["ssq"]).reshape(-1) for r in results])
    loss = np.sqrt(ssq.astype(np.float64)).sum() * CHF_TIK / B
    return np.float32(loss)
